# revision 18
# baseline (speedup 1.0000x reference)
"""Multi-head self-attention (B=1, S=4096, D=1024, H=16, DK=64) on 8 Trainium2
NeuronCores.

Sharding: tensor(model)-parallel over heads — 2 heads per core. Each core
computes Q^T/K^T/V^T for its 2 heads from the (host-pre-transposed) full x^T,
runs causal flash-style attention fully in transposed space (scores S^T with
keys on partitions, queries on the free dim; softmax sums come free via a
ones-column appended to V), normalizes O^T on the producer side (reciprocal of
the sums row broadcast across partitions via a rank-1 matmul), then exchanges
per-head normalized outputs in bf16 with AllToAlls so every core ends up with
all 16 heads' outputs for its own query rows, against which it runs the output
projection. Groups {0,1},{2,3},{4,5} exchange after chunks 1/3/5; the last two
chunks exchange individually so chunk 6's A2A overlaps chunk 7's attention and
the serial tail is only chunk 7's small A2A + half an emit.

The causal mask is structural (reference always builds jnp.tril), so the mask
input is not shipped to the device; masking is done with a precomputed
triangular tile on the diagonal blocks.
"""

import numpy as np
from contextlib import ExitStack

import concourse.bass as bass
import concourse.bacc as bacc
import concourse.tile as tile
import concourse.mybir as mybir
from concourse.bass_utils import run_bass_kernel_spmd
from concourse.masks import make_identity

F32 = mybir.dt.float32
F32R = mybir.dt.float32r
BF16 = mybir.dt.bfloat16
EXP = mybir.ActivationFunctionType.Exp

N_CORES = 8
D = 1024
H = 16
DK = 64        # head dim
HPC = H // N_CORES          # heads per core (2)
QC = 512                    # query-chunk width (free dim of S^T tiles)


def build(S=4096):
    """Build + compile the SPMD program (identical on all 8 cores)."""
    SC = S // QC            # query chunks (8)
    NSB = S // 128          # 128-wide seq blocks (32)
    QPER = S // N_CORES     # output rows per core (512)

    nc = bacc.Bacc("TRN2", target_bir_lowering=False, debug=False,
                   enable_asserts=False, num_devices=N_CORES)

    xt = nc.dram_tensor("xt", [D, S], BF16, kind="ExternalInput")
    wq = nc.dram_tensor("wq", [D, 128], BF16, kind="ExternalInput")
    wk = nc.dram_tensor("wk", [D, 128], BF16, kind="ExternalInput")
    wv = nc.dram_tensor("wv", [D, 128], BF16, kind="ExternalInput")
    wo = nc.dram_tensor("wo", [D, D], BF16, kind="ExternalInput")
    bq = nc.dram_tensor("bq", [128], F32, kind="ExternalInput")
    bk = nc.dram_tensor("bk", [128], F32, kind="ExternalInput")
    bv = nc.dram_tensor("bv", [128], F32, kind="ExternalInput")
    bo = nc.dram_tensor("bo", [D], BF16, kind="ExternalInput")
    out = nc.dram_tensor("out", [QPER, D], F32, kind="ExternalOutput")

    with tile.TileContext(nc) as tc, ExitStack() as ctx:
        sb = ctx.enter_context(tc.tile_pool(name="sb", bufs=1))
        sbx = ctx.enter_context(tc.tile_pool(name="sbx", bufs=2))
        sbpt = ctx.enter_context(tc.tile_pool(name="sbpt", bufs=3))
        sbtmp = ctx.enter_context(tc.tile_pool(name="sbtmp", bufs=3))
        sbn = ctx.enter_context(tc.tile_pool(name="sbn", bufs=2))
        # PSUM: one 3-slot pool of [128,1024] tiles (6 banks) shared by all
        # phases + a single [65,1024] accumulator tile (2 banks) = 8 banks.
        ps_big = ctx.enter_context(tc.tile_pool(name="ps_big", bufs=3, space="PSUM"))
        ps_ot = ctx.enter_context(tc.tile_pool(name="ps_ot", bufs=1, space="PSUM"))
        dram = ctx.enter_context(tc.tile_pool(name="dram", bufs=1, space="DRAM"))

        # ---- persistent tensors / constants ------------------------------
        wq_sb = sb.tile([128, 8, 128], BF16)
        wk_sb = sb.tile([128, 8, 128], BF16)
        wv_sb = sb.tile([128, 8, 128], BF16)
        nc.sync.dma_start(wq_sb[:], wq.ap().rearrange("(t p) m -> p t m", p=128))
        nc.sync.dma_start(wk_sb[:], wk.ap().rearrange("(t p) m -> p t m", p=128))
        nc.sync.dma_start(wv_sb[:], wv.ap().rearrange("(t p) m -> p t m", p=128))
        bq_sb = sb.tile([128, 1], F32)
        bk_sb = sb.tile([128, 1], F32)
        bv_sb = sb.tile([128, 1], F32)
        nc.sync.dma_start(bq_sb[:], bq.ap().rearrange("(p a) -> p a", a=1))
        nc.sync.dma_start(bk_sb[:], bk.ap().rearrange("(p a) -> p a", a=1))
        nc.sync.dma_start(bv_sb[:], bv.ap().rearrange("(p a) -> p a", a=1))
        bo_sb = sb.tile([1, D], BF16)
        wo_sb = sb.tile([128, 8, D], BF16)

        QT = sb.tile([128, S], BF16)      # rows 0-63 head0, 64-127 head1
        KT = sb.tile([128, S], BF16)
        # V' storage: per 128-seq block: [V_h0 (64) | 1 | V_h1 (64) | 1]
        Vp = sb.tile([128, NSB * 130], BF16)
        ones_col = sb.tile([128, 1], F32)
        nc.vector.memset(ones_col[:], 1.0)

        tri_f32 = sb.tile([128, 128], F32)  # tri[pj, j] = 1 if j >= pj else 0
        nc.gpsimd.memset(tri_f32[:], 1.0)
        nc.gpsimd.affine_select(
            out=tri_f32[:], in_=tri_f32[:], compare_op=mybir.AluOpType.is_ge,
            fill=0.0, base=0, pattern=[[1, 128]], channel_multiplier=-1)
        tri = sb.tile([128, 128], BF16)
        nc.vector.tensor_copy(tri[:], tri_f32[:])
        ident = sb.tile([128, 128], F32)
        make_identity(nc, ident[:])
        ones_row = sb.tile([1, 128], F32)
        nc.vector.memset(ones_row[:], 1.0)
        ones_sb = sb.tile([1, 128], BF16)
        nc.vector.tensor_copy(ones_sb[:], ones_row[:])

        # Output ownership: groups 0-2 span chunk pairs {0,1},{2,3},{4,5};
        # within group g rank r owns query cols [1024g+128r, 1024g+128(r+1)).
        # Chunks 6 and 7 exchange individually (64 queries per rank each) so
        # chunk 6's A2A overlaps chunk 7's attention.  Payload is the
        # producer-side-normalized O^T in bf16: rows (h*64+dk).
        a2a_in = [dram.tile([N_CORES, 128, 128], BF16, name=f"a2ain{g}")
                  for g in range(3)]
        a2a_out = [dram.tile([N_CORES, 128, 128], BF16, name=f"a2aout{g}")
                   for g in range(3)]
        a2a_in67 = [dram.tile([N_CORES, 128, 64], BF16, name=f"a2ain6{j}")
                    for j in range(2)]
        a2a_out67 = [dram.tile([N_CORES, 128, 64], BF16, name=f"a2aout6{j}")
                     for j in range(2)]

        # tiny warm-up exchange: absorbs the communicator-init barrier and
        # first-collective overhead while the early QKV chunks compute
        warm_in = dram.tile([N_CORES, 32], F32)
        warm_out = dram.tile([N_CORES, 32], F32)
        nc.gpsimd.collective_compute(
            "AllToAll", mybir.AluOpType.bypass,
            replica_groups=[list(range(N_CORES))],
            ins=[warm_in.opt()], outs=[warm_out.opt()])

        xt_r = xt.ap().rearrange("(t p) (c q) -> c p t q", p=128, q=QC)

        def make_qkv_bursts(c):
            """Per-chunk QKV work as small PE bursts. Interleaved between
            attention pairs of the previous chunk, they fill what would be
            PE idle time (keeping the HAM clock at 2.4 GHz)."""
            xt_sb = sbx.tile([128, 8, QC], BF16, tag="xt", name=f"xt{c}")
            nc.sync.dma_start(xt_sb[:], xt_r[c])
            cs = slice(c * QC, (c + 1) * QC)
            st8 = {}

            def proj_burst(w_sb, b_sb, dst):
                def run():
                    p_ps = ps_big.tile([128, 1024], F32, tag="st",
                                       name=f"qkv{c}_{dst.name}")
                    for t in range(8):
                        nc.tensor.matmul(p_ps[:, 0:512], w_sb[:, t, :],
                                         xt_sb[:, t, :],
                                         start=(t == 0), stop=(t == 7))
                    nc.vector.tensor_scalar_add(dst, p_ps[:, 0:512], b_sb[:])
                return run

            def q_burst():
                proj_burst(wq_sb, bq_sb, QT[:, cs])()
            def k_burst():
                proj_burst(wk_sb, bk_sb, KT[:, cs])()
            def v_burst():
                vt_sb = sbtmp.tile([128, QC], F32, tag="vt", name=f"vt{c}")
                st8["vt"] = vt_sb
                proj_burst(wv_sb, bv_sb, vt_sb[:])()

            def t_burst(sbk):
                def run():
                    blk = c * 4 + sbk
                    vt_sb = st8["vt"]
                    tp_ps = ps_big.tile([128, 128], F32, tag="st",
                                        name=f"tp{blk}")
                    nc.tensor.transpose(
                        tp_ps[:], vt_sb[:, sbk * 128:(sbk + 1) * 128], ident[:])
                    nc.vector.tensor_copy(Vp[:, blk * 130: blk * 130 + 64],
                                          tp_ps[:, 0:64])
                    nc.vector.tensor_copy(Vp[:, blk * 130 + 65: blk * 130 + 129],
                                          tp_ps[:, 64:128])
                    nc.vector.tensor_copy(Vp[:, blk * 130 + 64: blk * 130 + 65],
                                          ones_col[:])
                    nc.vector.tensor_copy(Vp[:, blk * 130 + 129: blk * 130 + 130],
                                          ones_col[:])
                return run

            return [q_burst, k_burst, v_burst,
                    t_burst(0), t_burst(1), t_burst(2), t_burst(3)]

        of_tiles = {}

        def make_emit_bursts(key, src_list, w, out_row0):
            """Output projection for one gathered O^T tile as PE bursts.
            src_list: list of (a2a_out tile, col offset) to gather first.
            w: stationary width (queries per emit). out_row0: first output
            row of this emit in the core's [QPER, D] output shard."""
            def gather():
                of_sb = sbn.tile([128, 8, 128], BF16, tag="of", name=f"of{key}")
                of_tiles[key] = of_sb
                for src, co in src_list:
                    nc.sync.dma_start(
                        of_sb[:, :, co:co + w],
                        src[:, :, :].rearrange("s p q -> p s q"))

            def proj(n2, co):
                def run():
                    of_sb = of_tiles[key]
                    op_ps = ps_big.tile([128, 1024], F32, tag="st",
                                        name=f"op{key}_{n2}")
                    for s in range(8):
                        nc.tensor.matmul(
                            op_ps[0:w, 0:512], of_sb[:, s, co:co + w],
                            wo_sb[:, s, n2 * 512:(n2 + 1) * 512],
                            start=(s == 0), stop=False)
                    nc.tensor.matmul(op_ps[0:w, 0:512], ones_sb[0:1, 0:w],
                                     bo_sb[0:1, n2 * 512:(n2 + 1) * 512],
                                     start=False, stop=True)
                    o_sb = sbtmp.tile([128, 512], F32, tag="osb",
                                      name=f"o{key}_{n2}")
                    nc.vector.tensor_copy(o_sb[0:w, :], op_ps[0:w, 0:512])
                    nc.sync.dma_start(
                        out.ap()[out_row0:out_row0 + w,
                                 n2 * 512:(n2 + 1) * 512],
                        o_sb[0:w, :])
                return run

            co = 0 if w == 128 else src_list[0][1]
            return [gather], [proj(0, co), proj(1, co)]

        for b in make_qkv_bursts(0):
            b()
        for c in range(SC):
            pending = []
            tail_b = []
            # emit bursts for groups whose A2A has had >=1.5 chunks to land;
            # gathers go first (DMA issue only), projections at the end
            if c == 3:
                g_b, p_b = make_emit_bursts(0, [(a2a_out[0], 0)], 128, 0)
                pending += g_b; tail_b += p_b
            if c == 5:
                g_b, p_b = make_emit_bursts(1, [(a2a_out[1], 0)], 128, 128)
                pending += g_b; tail_b += p_b
            if c == 7:
                g_b, p_b = make_emit_bursts(2, [(a2a_out[2], 0)], 128, 256)
                pending += g_b; tail_b += p_b
                g_b, p_b = make_emit_bursts(60, [(a2a_out67[0], 0)], 64, 384)
                pending += g_b; tail_b += p_b
            pending += make_qkv_bursts(c + 1) if c + 1 < SC else []
            pending += tail_b
            nb = len(pending)
            done = 0

            # ---- causal attention for chunk c, both heads ----------------
            cs = slice(c * QC, (c + 1) * QC)
            nkb = 4 * (c + 1)
            npairs = nkb // 2
            ot = ps_ot.tile([65, 1024], F32, tag="ot", name=f"ot{c}")
            ots = [ot[:, 0:512], ot[:, 512:1024]]
            for p, kbp in enumerate(range(0, nkb, 2)):
                st_h = [ps_big.tile([128, 1024], F32, tag="st",
                                    name=f"st{c}_{kbp}_{h}") for h in range(2)]
                # heads interleaved: their PE row-groups (0-63 / 64-127)
                # execute concurrently in the array
                for j in range(2):
                    kb = kbp + j
                    for h in range(2):
                        hs = slice(h * 64, (h + 1) * 64)
                        nc.tensor.matmul(
                            st_h[h][:, j * 512:(j + 1) * 512],
                            KT[hs, kb * 128:(kb + 1) * 128],
                            QT[hs, cs], start=True, stop=True)
                pt_h = []
                for h in range(2):
                    pt = sbpt.tile([128, 1024], BF16, tag="pt",
                                   name=f"pt{c}_{kbp}_{h}")
                    nc.scalar.activation(pt[:], st_h[h][:], EXP, scale=0.125)
                    for j in range(2):
                        t = kbp + j - 4 * c
                        if t >= 0:   # diagonal block: apply causal mask
                            ms = slice(j * 512 + 128 * t, j * 512 + 128 * t + 128)
                            nc.vector.tensor_mul(pt[:, ms], pt[:, ms], tri[:])
                    pt_h.append(pt)
                for h in range(2):
                    for j in range(2):
                        kb = kbp + j
                        t = kb - 4 * c
                        off = 128 * t if t > 0 else 0  # fully-masked cols skipped
                        nc.tensor.matmul(
                            ots[h][:, off:512],
                            Vp[:, kb * 130 + h * 65: kb * 130 + (h + 1) * 65],
                            pt_h[h][:, j * 512 + off:(j + 1) * 512],
                            start=(kb == 0), stop=(kb == nkb - 1))
                # spread next chunk's QKV + emit bursts across this chunk's
                # pairs
                want = (p + 1) * nb // npairs
                while done < want:
                    pending[done]()
                    done += 1
            while done < nb:
                pending[done]()
                done += 1
            if c == 1:
                nc.sync.dma_start(bo_sb[:], bo.ap().rearrange("(a n) -> a n", a=1))
                nc.sync.dma_start(wo_sb[:],
                                  wo.ap().rearrange("(t p) n -> p t n", p=128))

            # ---- producer-side softmax normalization ---------------------
            # rinv = 1/sums (row 64); broadcast across the 64 dk partitions
            # with a rank-1 matmul; normalized O^T in bf16 is the A2A payload.
            sums = sbtmp.tile([1, 1024], F32, tag="sums", name=f"sums{c}")
            nc.vector.tensor_copy(sums[:], ot[64:65, :])
            rinv = sbtmp.tile([1, 1024], F32, tag="rinv", name=f"rinv{c}")
            nc.vector.reciprocal_approx_fast(rinv[:], sums[:])
            rinv_b = sbtmp.tile([1, 1024], BF16, tag="rinvb", name=f"rinvb{c}")
            nc.vector.tensor_copy(rinv_b[:], rinv[:])
            bc = ps_big.tile([128, 1024], F32, tag="st", name=f"bc{c}")
            for half in range(2):
                nc.tensor.matmul(bc[0:64, half * 512:(half + 1) * 512],
                                 ones_sb[0:1, 0:64],
                                 rinv_b[0:1, half * 512:(half + 1) * 512],
                                 start=True, stop=True)
            bc_sb = sbtmp.tile([64, 1024], F32, tag="bcast", name=f"bcsb{c}")
            nc.vector.tensor_copy(bc_sb[:], bc[0:64, :])
            norm = sbn.tile([64, 1024], BF16, tag="norm", name=f"norm{c}")
            nc.vector.tensor_mul(norm[:], ot[0:64, :], bc_sb[:])

            # ---- stage into the A2A buffer -------------------------------
            norm_h = norm[:].rearrange("p (h q) -> p h q", h=2)
            if c < 6:
                g, jj = c // 2, c % 2
                for i in range(4):
                    dst = 4 * jj + i
                    nc.sync.dma_start(
                        a2a_in[g][dst, :, :].rearrange("(h p) q -> p h q", h=2),
                        norm_h[:, :, i * 128:(i + 1) * 128])
                if jj == 1:
                    nc.gpsimd.collective_compute(
                        "AllToAll", mybir.AluOpType.bypass,
                        replica_groups=[list(range(N_CORES))],
                        ins=[a2a_in[g].opt()], outs=[a2a_out[g].opt()])
            else:
                j67 = c - 6
                for dst in range(8):
                    nc.sync.dma_start(
                        a2a_in67[j67][dst, :, :].rearrange("(h p) q -> p h q",
                                                           h=2),
                        norm_h[:, :, dst * 64:(dst + 1) * 64])
                nc.gpsimd.collective_compute(
                    "AllToAll", mybir.AluOpType.bypass,
                    replica_groups=[list(range(N_CORES))],
                    ins=[a2a_in67[j67].opt()], outs=[a2a_out67[j67].opt()])

        # tail: only chunk 7's exchange + its half-emit remain
        g_b, p_b = make_emit_bursts(61, [(a2a_out67[1], 64)], 64, 448)
        for b in g_b + p_b:
            b()

    nc.compile()
    return nc


_NC_CACHE = {}


def _get_nc(S):
    if S not in _NC_CACHE:
        _NC_CACHE[S] = build(S)
    return _NC_CACHE[S]


def kernel(x, mask, Wq, bq, Wk, bk, Wv, bv, Wo, bo):
    import ml_dtypes
    x = np.asarray(x, np.float32)
    S = x.shape[1]
    xt = np.ascontiguousarray(x[0].T).astype(ml_dtypes.bfloat16)  # [D, S]
    Wq, Wk, Wv, Wo = (np.asarray(w, np.float32) for w in (Wq, Wk, Wv, Wo))
    bq, bk, bv, bo = (np.asarray(b, np.float32) for b in (bq, bk, bv, bo))
    # mask is structurally causal (jnp.tril in the reference); handled on-device.

    in_maps = []
    for r in range(N_CORES):
        sl = slice(128 * r, 128 * (r + 1))
        in_maps.append({
            "xt": xt,
            "wq": np.ascontiguousarray(Wq[:, sl]).astype(ml_dtypes.bfloat16),
            "wk": np.ascontiguousarray(Wk[:, sl]).astype(ml_dtypes.bfloat16),
            "wv": np.ascontiguousarray(Wv[:, sl]).astype(ml_dtypes.bfloat16),
            "wo": Wo.astype(ml_dtypes.bfloat16),
            "bq": np.ascontiguousarray(bq[sl]),
            "bk": np.ascontiguousarray(bk[sl]),
            "bv": np.ascontiguousarray(bv[sl]),
            "bo": bo.astype(ml_dtypes.bfloat16),
        })
    nc = _get_nc(S)
    global LAST_RESULT
    LAST_RESULT = run_bass_kernel_spmd(nc, in_maps, list(range(N_CORES)),
                                       trace=TRACE)
    res = LAST_RESULT.results
    full = np.empty((S, D), np.float32)
    for r in range(N_CORES):
        o = res[r]["out"]                       # [512, D]
        for g in range(3):
            full[1024 * g + 128 * r: 1024 * g + 128 * r + 128] = \
                o[128 * g: 128 * g + 128]
        full[3072 + 64 * r: 3072 + 64 * r + 64] = o[384:448]
        full[3584 + 64 * r: 3584 + 64 * r + 64] = o[448:512]
    return full[None].astype(np.float32)


TRACE = False          # test harness flips this to profile
LAST_RESULT = None


# revision 46
# speedup vs baseline: 1.0673x; 1.0673x over previous
"""Multi-head self-attention (B=1, S=4096, D=1024, H=16, DK=64) on 8 Trainium2
NeuronCores.

Sharding: tensor(model)-parallel over heads — 2 heads per core. Each core
computes Q^T/K^T/V^T for its 2 heads from the (host-pre-transposed) full x^T,
runs causal flash-style attention fully in transposed space (scores S^T with
keys on partitions, queries on the free dim; softmax sums come free via a
ones-column appended to V), normalizes O^T on the producer side (reciprocal of
the sums row broadcast across partitions via a rank-1 matmul), then exchanges
per-head normalized outputs in bf16 with AllToAlls so every core ends up with
all 16 heads' outputs for its own query rows, against which it runs the output
projection. Groups {0,1},{2,3},{4,5} exchange after chunks 1/3/5; the last two
chunks exchange individually so chunk 6's A2A overlaps chunk 7's attention and
the serial tail is only chunk 7's small A2A + half an emit.

The causal mask is structural (reference always builds jnp.tril), so the mask
input is not shipped to the device; masking is done with a precomputed
triangular tile on the diagonal blocks.
"""

import numpy as np
from contextlib import ExitStack

import concourse.bass as bass
import concourse.bacc as bacc
import concourse.tile as tile
import concourse.mybir as mybir
from concourse.bass_utils import run_bass_kernel_spmd
from concourse.masks import make_identity

F32 = mybir.dt.float32
F32R = mybir.dt.float32r
BF16 = mybir.dt.bfloat16
FP8 = mybir.dt.float8e4
DR = mybir.MatmulPerfMode.DoubleRow
EXP = mybir.ActivationFunctionType.Exp

N_CORES = 8
D = 1024
H = 16
DK = 64        # head dim
HPC = H // N_CORES          # heads per core (2)
QC = 512                    # query-chunk width (free dim of S^T tiles)


def build(S=4096):
    """Build + compile the SPMD program (identical on all 8 cores)."""
    SC = S // QC            # query chunks (8)
    NSB = S // 128          # 128-wide seq blocks (32)
    QPER = S // N_CORES     # output rows per core (512)

    nc = bacc.Bacc("TRN2", target_bir_lowering=False, debug=False,
                   enable_asserts=False, num_devices=N_CORES)

    xt = nc.dram_tensor("xt", [D, S], BF16, kind="ExternalInput")
    wq = nc.dram_tensor("wq", [D, 128], BF16, kind="ExternalInput")
    wk = nc.dram_tensor("wk", [D, 128], BF16, kind="ExternalInput")
    wv = nc.dram_tensor("wv", [D, 128], BF16, kind="ExternalInput")
    wo = nc.dram_tensor("wo", [D, D], BF16, kind="ExternalInput")
    bq = nc.dram_tensor("bq", [128], F32, kind="ExternalInput")
    bk = nc.dram_tensor("bk", [128], F32, kind="ExternalInput")
    bv = nc.dram_tensor("bv", [128], F32, kind="ExternalInput")
    bo = nc.dram_tensor("bo", [D], BF16, kind="ExternalInput")
    out = nc.dram_tensor("out", [QPER, D], F32, kind="ExternalOutput")

    with tile.TileContext(nc) as tc, ExitStack() as ctx:
        sb = ctx.enter_context(tc.tile_pool(name="sb", bufs=1))
        sbx = ctx.enter_context(tc.tile_pool(name="sbx", bufs=2))
        sbpt = ctx.enter_context(tc.tile_pool(name="sbpt", bufs=3))
        sbtmp = ctx.enter_context(tc.tile_pool(name="sbtmp", bufs=3))
        sbn = ctx.enter_context(tc.tile_pool(name="sbn", bufs=2))
        # PSUM: one 3-slot pool of [128,1024] tiles (6 banks) shared by all
        # phases + a single [65,1024] accumulator tile (2 banks) = 8 banks.
        ps_big = ctx.enter_context(tc.tile_pool(name="ps_big", bufs=3, space="PSUM"))
        ps_ot = ctx.enter_context(tc.tile_pool(name="ps_ot", bufs=1, space="PSUM"))
        dram = ctx.enter_context(tc.tile_pool(name="dram", bufs=1, space="DRAM"))

        # ---- persistent tensors / constants ------------------------------
        wq_sb = sb.tile([128, 8, 128], BF16)
        wk_sb = sb.tile([128, 8, 128], BF16)
        wv_sb = sb.tile([128, 8, 128], BF16)
        nc.sync.dma_start(wq_sb[:], wq.ap().rearrange("(t p) m -> p t m", p=128))
        nc.sync.dma_start(wk_sb[:], wk.ap().rearrange("(t p) m -> p t m", p=128))
        nc.sync.dma_start(wv_sb[:], wv.ap().rearrange("(t p) m -> p t m", p=128))
        bq_sb = sb.tile([128, 1], F32)
        bk_sb = sb.tile([128, 1], F32)
        bv_sb = sb.tile([128, 1], F32)
        nc.sync.dma_start(bq_sb[:], bq.ap().rearrange("(p a) -> p a", a=1))
        nc.sync.dma_start(bk_sb[:], bk.ap().rearrange("(p a) -> p a", a=1))
        nc.sync.dma_start(bv_sb[:], bv.ap().rearrange("(p a) -> p a", a=1))
        bo_sb = sb.tile([1, D], BF16)
        wo_sb = sb.tile([128, 8, D], BF16)

        QT = sb.tile([128, S], BF16)      # rows 0-63 head0, 64-127 head1
        KT = sb.tile([128, S], BF16)
        # V' storage: per 128-seq block: [V_h0 (64) | 1 | V_h1 (64) | 1]
        Vp = sb.tile([128, NSB * 130], BF16)
        ones_col = sb.tile([128, 1], F32)
        nc.vector.memset(ones_col[:], 1.0)

        tri_f32 = sb.tile([128, 128], F32)  # tri[pj, j] = 1 if j >= pj else 0
        nc.gpsimd.memset(tri_f32[:], 1.0)
        nc.gpsimd.affine_select(
            out=tri_f32[:], in_=tri_f32[:], compare_op=mybir.AluOpType.is_ge,
            fill=0.0, base=0, pattern=[[1, 128]], channel_multiplier=-1)
        tri = sb.tile([128, 128], BF16)
        nc.vector.tensor_copy(tri[:], tri_f32[:])
        ident = sb.tile([128, 128], F32)
        make_identity(nc, ident[:])
        ones_row = sb.tile([1, 128], F32)
        nc.vector.memset(ones_row[:], 1.0)
        ones_sb = sb.tile([1, 128], BF16)
        nc.vector.tensor_copy(ones_sb[:], ones_row[:])

        # Output ownership: groups 0-2 span chunk pairs {0,1},{2,3},{4,5};
        # within group g rank r owns query cols [1024g+128r, 1024g+128(r+1)).
        # Chunks 6 and 7 exchange individually (64 queries per rank each) so
        # chunk 6's A2A overlaps chunk 7's attention.  Payload is the
        # producer-side-normalized O^T in bf16: rows (h*64+dk).
        a2a_in = [dram.tile([N_CORES, 128, 128], BF16, name=f"a2ain{g}")
                  for g in range(3)]
        a2a_out = [dram.tile([N_CORES, 128, 128], BF16, name=f"a2aout{g}")
                   for g in range(3)]
        a2a_in67 = [dram.tile([N_CORES, 128, 64], BF16, name=f"a2ain6{j}")
                    for j in range(2)]
        a2a_out67 = [dram.tile([N_CORES, 128, 64], BF16, name=f"a2aout6{j}")
                     for j in range(2)]

        # tiny warm-up exchange: absorbs the communicator-init barrier and
        # first-collective overhead while the early QKV chunks compute
        warm_in = dram.tile([N_CORES, 32], F32)
        warm_out = dram.tile([N_CORES, 32], F32)
        nc.gpsimd.collective_compute(
            "AllToAll", mybir.AluOpType.bypass,
            replica_groups=[list(range(N_CORES))],
            ins=[warm_in.opt()], outs=[warm_out.opt()])

        xt_r = xt.ap().rearrange("(t p) (c q) -> c p t q", p=128, q=QC)

        def make_qkv_bursts(c):
            """Per-chunk QKV work as small PE bursts. Interleaved between
            attention pairs of the previous chunk, they fill what would be
            PE idle time (keeping the HAM clock at 2.4 GHz)."""
            xt_sb = sbx.tile([128, 8, QC], BF16, tag="xt", name=f"xt{c}")
            nc.sync.dma_start(xt_sb[:], xt_r[c])
            cs = slice(c * QC, (c + 1) * QC)
            st8 = {}

            def proj_burst(w_sb, b_sb, dst):
                def run():
                    p_ps = ps_big.tile([128, 1024], F32, tag="st",
                                       name=f"qkv{c}_{dst.name}")
                    for t in range(8):
                        nc.tensor.matmul(p_ps[:, 0:512], w_sb[:, t, :],
                                         xt_sb[:, t, :],
                                         start=(t == 0), stop=(t == 7))
                    nc.vector.tensor_scalar_add(dst, p_ps[:, 0:512], b_sb[:])
                return run

            def q_burst():
                proj_burst(wq_sb, bq_sb, QT[:, cs])()
            def k_burst():
                proj_burst(wk_sb, bk_sb, KT[:, cs])()
            def v_burst():
                vt_sb = sbtmp.tile([128, QC], F32, tag="vt", name=f"vt{c}")
                st8["vt"] = vt_sb
                proj_burst(wv_sb, bv_sb, vt_sb[:])()

            def t_burst(sbk):
                def run():
                    blk = c * 4 + sbk
                    vt_sb = st8["vt"]
                    tp_ps = ps_big.tile([128, 128], F32, tag="st",
                                        name=f"tp{blk}")
                    nc.tensor.transpose(
                        tp_ps[:], vt_sb[:, sbk * 128:(sbk + 1) * 128], ident[:])
                    nc.vector.tensor_copy(Vp[:, blk * 130: blk * 130 + 64],
                                          tp_ps[:, 0:64])
                    nc.vector.tensor_copy(Vp[:, blk * 130 + 65: blk * 130 + 129],
                                          tp_ps[:, 64:128])
                    nc.vector.tensor_copy(Vp[:, blk * 130 + 64: blk * 130 + 65],
                                          ones_col[:])
                    nc.vector.tensor_copy(Vp[:, blk * 130 + 129: blk * 130 + 130],
                                          ones_col[:])
                return run

            return [q_burst, k_burst, v_burst,
                    t_burst(0), t_burst(1), t_burst(2), t_burst(3)]

        of_tiles = {}

        def make_emit_bursts(key, src_list, w, out_row0):
            """Output projection for one gathered O^T tile as PE bursts.
            src_list: list of (a2a_out tile, col offset) to gather first.
            w: stationary width (queries per emit). out_row0: first output
            row of this emit in the core's [QPER, D] output shard."""
            def gather():
                of_sb = sbn.tile([128, 8, 128], BF16, tag="of", name=f"of{key}")
                of_tiles[key] = of_sb
                for src, co in src_list:
                    nc.sync.dma_start(
                        of_sb[:, :, co:co + w],
                        src[:, :, :].rearrange("s p q -> p s q"))

            def proj(n2, co):
                def run():
                    of_sb = of_tiles[key]
                    op_ps = ps_big.tile([128, 1024], F32, tag="st",
                                        name=f"op{key}_{n2}")
                    for s in range(8):
                        nc.tensor.matmul(
                            op_ps[0:w, 0:512], of_sb[:, s, co:co + w],
                            wo_sb[:, s, n2 * 512:(n2 + 1) * 512],
                            start=(s == 0), stop=False)
                    nc.tensor.matmul(op_ps[0:w, 0:512], ones_sb[0:1, 0:w],
                                     bo_sb[0:1, n2 * 512:(n2 + 1) * 512],
                                     start=False, stop=True)
                    o_sb = sbtmp.tile([128, 512], F32, tag="osb",
                                      name=f"o{key}_{n2}")
                    nc.vector.tensor_copy(o_sb[0:w, :], op_ps[0:w, 0:512])
                    nc.sync.dma_start(
                        out.ap()[out_row0:out_row0 + w,
                                 n2 * 512:(n2 + 1) * 512],
                        o_sb[0:w, :])
                return run

            co = 0 if w == 128 else src_list[0][1]
            return [gather], [proj(0, co), proj(1, co)]

        for b in make_qkv_bursts(0):
            b()
        for c in range(SC):
            pending = []
            tail_b = []
            # emit bursts for groups whose A2A has had >=1.5 chunks to land;
            # gathers go first (DMA issue only), projections at the end
            if c == 3:
                g_b, p_b = make_emit_bursts(0, [(a2a_out[0], 0)], 128, 0)
                pending += g_b; tail_b += p_b
            if c == 5:
                g_b, p_b = make_emit_bursts(1, [(a2a_out[1], 0)], 128, 128)
                pending += g_b; tail_b += p_b
            if c == 7:
                g_b, p_b = make_emit_bursts(2, [(a2a_out[2], 0)], 128, 256)
                pending += g_b; tail_b += p_b
                g_b, p_b = make_emit_bursts(60, [(a2a_out67[0], 0)], 64, 384)
                pending += g_b; tail_b += p_b
            pending += make_qkv_bursts(c + 1) if c + 1 < SC else []
            pending += tail_b
            nb = len(pending)
            done = 0

            # ---- causal attention for chunk c, both heads ----------------
            cs = slice(c * QC, (c + 1) * QC)
            nkb = 4 * (c + 1)
            npairs = nkb // 2
            ot = ps_ot.tile([65, 1024], F32, tag="ot", name=f"ot{c}")
            ots = [ot[:, 0:512], ot[:, 512:1024]]
            for p, kbp in enumerate(range(0, nkb, 2)):
                st_h = [ps_big.tile([128, 1024], F32, tag="st",
                                    name=f"st{c}_{kbp}_{h}") for h in range(2)]
                # heads interleaved: their PE row-groups (0-63 / 64-127)
                # execute concurrently in the array
                for j in range(2):
                    kb = kbp + j
                    for h in range(2):
                        hs = slice(h * 64, (h + 1) * 64)
                        nc.tensor.matmul(
                            st_h[h][:, j * 512:(j + 1) * 512],
                            KT[hs, kb * 128:(kb + 1) * 128],
                            QT[hs, cs], start=True, stop=True)
                pt_h = []
                for h in range(2):
                    pt = sbpt.tile([128, 1024], BF16, tag="pt",
                                   name=f"pt{c}_{kbp}_{h}")
                    nc.scalar.activation(pt[:], st_h[h][:], EXP, scale=0.125)
                    for j in range(2):
                        t = kbp + j - 4 * c
                        if t >= 0:   # diagonal block: apply causal mask
                            ms = slice(j * 512 + 128 * t, j * 512 + 128 * t + 128)
                            nc.vector.tensor_mul(pt[:, ms], pt[:, ms], tri[:])
                    pt_h.append(pt)
                for h in range(2):
                    for j in range(2):
                        kb = kbp + j
                        t = kb - 4 * c
                        off = 128 * t if t > 0 else 0  # fully-masked cols skipped
                        nc.tensor.matmul(
                            ots[h][:, off:512],
                            Vp[:, kb * 130 + h * 65: kb * 130 + (h + 1) * 65],
                            pt_h[h][:, j * 512 + off:(j + 1) * 512],
                            start=(kb == 0), stop=(kb == nkb - 1))
                # spread next chunk's QKV + emit bursts across this chunk's
                # pairs
                want = (p + 1) * nb // npairs
                while done < want:
                    pending[done]()
                    done += 1
            while done < nb:
                pending[done]()
                done += 1
            if c == 1:
                nc.sync.dma_start(bo_sb[:], bo.ap().rearrange("(a n) -> a n", a=1))
                nc.sync.dma_start(wo_sb[:],
                                  wo.ap().rearrange("(t p) n -> p t n", p=128))

            # ---- producer-side softmax normalization ---------------------
            # rinv = 1/sums (row 64); broadcast across the 64 dk partitions
            # with a rank-1 matmul; normalized O^T in bf16 is the A2A payload.
            # copy ot to SBUF first: frees the PSUM accumulator (~2.4us) so
            # the next chunk's AV matmuls never wait on the norm chain.
            # NOTE: the custom-DVE reciprocal requires a partition-0 SBUF
            # input tile; feeding it a base_partition=64 slice breaks it.
            onsb = sbtmp.tile([64, 1024], F32, tag="onsb", name=f"onsb{c}")
            nc.vector.tensor_copy(onsb[:], ot[0:64, :])
            sums = sbtmp.tile([1, 1024], F32, tag="sums", name=f"sums{c}")
            nc.vector.tensor_copy(sums[:], ot[64:65, :])
            rinv = sbtmp.tile([1, 1024], F32, tag="rinv", name=f"rinv{c}")
            nc.vector.reciprocal_approx_fast(rinv[:], sums[:])
            rinv_b = sbtmp.tile([1, 1024], BF16, tag="rinvb", name=f"rinvb{c}")
            nc.vector.tensor_copy(rinv_b[:], rinv[:])
            bc = ps_big.tile([128, 1024], F32, tag="st", name=f"bc{c}")
            for half in range(2):
                nc.tensor.matmul(bc[0:64, half * 512:(half + 1) * 512],
                                 ones_sb[0:1, 0:64],
                                 rinv_b[0:1, half * 512:(half + 1) * 512],
                                 start=True, stop=True)
            norm = sbn.tile([64, 1024], BF16, tag="norm", name=f"norm{c}")
            nc.vector.tensor_mul(norm[:], bc[0:64, :], onsb[:])

            # ---- stage into the A2A buffer -------------------------------
            norm_h = norm[:].rearrange("p (h q) -> p h q", h=2)
            if c < 6:
                g, jj = c // 2, c % 2
                for i in range(4):
                    dst = 4 * jj + i
                    nc.sync.dma_start(
                        a2a_in[g][dst, :, :].rearrange("(h p) q -> p h q", h=2),
                        norm_h[:, :, i * 128:(i + 1) * 128])
                if jj == 1:
                    nc.gpsimd.collective_compute(
                        "AllToAll", mybir.AluOpType.bypass,
                        replica_groups=[list(range(N_CORES))],
                        ins=[a2a_in[g].opt()], outs=[a2a_out[g].opt()])
            else:
                j67 = c - 6
                for dst in range(8):
                    nc.sync.dma_start(
                        a2a_in67[j67][dst, :, :].rearrange("(h p) q -> p h q",
                                                           h=2),
                        norm_h[:, :, dst * 64:(dst + 1) * 64])
                nc.gpsimd.collective_compute(
                    "AllToAll", mybir.AluOpType.bypass,
                    replica_groups=[list(range(N_CORES))],
                    ins=[a2a_in67[j67].opt()], outs=[a2a_out67[j67].opt()])

        # tail: only chunk 7's exchange + its half-emit remain
        g_b, p_b = make_emit_bursts(61, [(a2a_out67[1], 64)], 64, 448)
        for b in g_b + p_b:
            b()

    nc.compile()
    return nc


_NC_CACHE = {}


def _get_nc(S):
    if S not in _NC_CACHE:
        _NC_CACHE[S] = build(S)
    return _NC_CACHE[S]


def kernel(x, mask, Wq, bq, Wk, bk, Wv, bv, Wo, bo):
    import ml_dtypes
    x = np.asarray(x, np.float32)
    S = x.shape[1]
    xt = np.ascontiguousarray(x[0].T).astype(ml_dtypes.bfloat16)  # [D, S]
    Wq, Wk, Wv, Wo = (np.asarray(w, np.float32) for w in (Wq, Wk, Wv, Wo))
    bq, bk, bv, bo = (np.asarray(b, np.float32) for b in (bq, bk, bv, bo))
    # mask is structurally causal (jnp.tril in the reference); handled on-device.

    in_maps = []
    for r in range(N_CORES):
        sl = slice(128 * r, 128 * (r + 1))
        in_maps.append({
            "xt": xt,
            "wq": np.ascontiguousarray(Wq[:, sl]).astype(ml_dtypes.bfloat16),
            "wk": np.ascontiguousarray(Wk[:, sl]).astype(ml_dtypes.bfloat16),
            "wv": np.ascontiguousarray(Wv[:, sl]).astype(ml_dtypes.bfloat16),
            "wo": Wo.astype(ml_dtypes.bfloat16),
            "bq": np.ascontiguousarray(bq[sl]),
            "bk": np.ascontiguousarray(bk[sl]),
            "bv": np.ascontiguousarray(bv[sl]),
            "bo": bo.astype(ml_dtypes.bfloat16),
        })
    nc = _get_nc(S)
    global LAST_RESULT
    LAST_RESULT = run_bass_kernel_spmd(nc, in_maps, list(range(N_CORES)),
                                       trace=TRACE)
    res = LAST_RESULT.results
    full = np.empty((S, D), np.float32)
    for r in range(N_CORES):
        o = res[r]["out"]                       # [512, D]
        for g in range(3):
            full[1024 * g + 128 * r: 1024 * g + 128 * r + 128] = \
                o[128 * g: 128 * g + 128]
        full[3072 + 64 * r: 3072 + 64 * r + 64] = o[384:448]
        full[3584 + 64 * r: 3584 + 64 * r + 64] = o[448:512]
    return full[None].astype(np.float32)


TRACE = False          # test harness flips this to profile
LAST_RESULT = None


# revision 56
# speedup vs baseline: 1.1048x; 1.0351x over previous
"""Multi-head self-attention (B=1, S=4096, D=1024, H=16, DK=64) on 8 Trainium2
NeuronCores.

Sharding: tensor(model)-parallel over heads — 2 heads per core. Each core
computes Q^T/K^T/V^T for its 2 heads from the (host-pre-transposed) full x^T,
runs causal flash-style attention fully in transposed space (scores S^T with
keys on partitions, queries on the free dim; softmax sums come free via a
ones-column appended to V), normalizes O^T on the producer side (reciprocal of
the sums row broadcast across partitions via a rank-1 matmul), then exchanges
per-head normalized outputs in bf16 with AllToAlls so every core ends up with
all 16 heads' outputs for its own query rows, against which it runs the output
projection. Groups {0,1},{2,3},{4,5} exchange after chunks 1/3/5; the last two
chunks exchange individually so chunk 6's A2A overlaps chunk 7's attention and
the serial tail is only chunk 7's small A2A + half an emit.

The causal mask is structural (reference always builds jnp.tril), so the mask
input is not shipped to the device; masking is done with a precomputed
triangular tile on the diagonal blocks.
"""

import numpy as np
from contextlib import ExitStack

import concourse.bass as bass
import concourse.bacc as bacc
import concourse.tile as tile
import concourse.mybir as mybir
from concourse.bass_utils import run_bass_kernel_spmd
from concourse.masks import make_identity

F32 = mybir.dt.float32
F32R = mybir.dt.float32r
BF16 = mybir.dt.bfloat16
FP8 = mybir.dt.float8e4
DR = mybir.MatmulPerfMode.DoubleRow
EXP = mybir.ActivationFunctionType.Exp

N_CORES = 8
D = 1024
H = 16
DK = 64        # head dim
HPC = H // N_CORES          # heads per core (2)
QC = 512                    # query-chunk width (free dim of S^T tiles)


def build(S=4096):
    """Build + compile the SPMD program (identical on all 8 cores)."""
    SC = S // QC            # query chunks (8)
    NSB = S // 128          # 128-wide seq blocks (32)
    QPER = S // N_CORES     # output rows per core (512)

    nc = bacc.Bacc("TRN2", target_bir_lowering=False, debug=False,
                   enable_asserts=False, num_devices=N_CORES)

    xt = nc.dram_tensor("xt", [D, S], BF16, kind="ExternalInput")
    wq = nc.dram_tensor("wq", [D, 128], BF16, kind="ExternalInput")
    wk = nc.dram_tensor("wk", [D, 128], BF16, kind="ExternalInput")
    wv = nc.dram_tensor("wv", [D, 128], BF16, kind="ExternalInput")
    wo = nc.dram_tensor("wo", [D, D], BF16, kind="ExternalInput")
    bq = nc.dram_tensor("bq", [128], F32, kind="ExternalInput")
    bk = nc.dram_tensor("bk", [128], F32, kind="ExternalInput")
    bv = nc.dram_tensor("bv", [128], F32, kind="ExternalInput")
    bo = nc.dram_tensor("bo", [D], BF16, kind="ExternalInput")
    out = nc.dram_tensor("out", [QPER, D], F32, kind="ExternalOutput")

    with tile.TileContext(nc) as tc, ExitStack() as ctx:
        sb = ctx.enter_context(tc.tile_pool(name="sb", bufs=1))
        sbx = ctx.enter_context(tc.tile_pool(name="sbx", bufs=2))
        sbpt = ctx.enter_context(tc.tile_pool(name="sbpt", bufs=3))
        sbtmp = ctx.enter_context(tc.tile_pool(name="sbtmp", bufs=6))
        sbn = ctx.enter_context(tc.tile_pool(name="sbn", bufs=2))
        # PSUM: one 3-slot pool of [128,1024] tiles (6 banks) shared by all
        # phases + a single [65,1024] accumulator tile (2 banks) = 8 banks.
        ps_big = ctx.enter_context(tc.tile_pool(name="ps_big", bufs=3, space="PSUM"))
        ps_ot = ctx.enter_context(tc.tile_pool(name="ps_ot", bufs=1, space="PSUM"))
        dram = ctx.enter_context(tc.tile_pool(name="dram", bufs=1, space="DRAM"))

        # ---- persistent tensors / constants ------------------------------
        wq_sb = sb.tile([128, 8, 128], BF16)
        wk_sb = sb.tile([128, 8, 128], BF16)
        wv_sb = sb.tile([128, 8, 128], BF16)
        nc.sync.dma_start(wq_sb[:], wq.ap().rearrange("(t p) m -> p t m", p=128))
        nc.sync.dma_start(wk_sb[:], wk.ap().rearrange("(t p) m -> p t m", p=128))
        nc.sync.dma_start(wv_sb[:], wv.ap().rearrange("(t p) m -> p t m", p=128))
        bq_sb = sb.tile([128, 1], F32)
        bk_sb = sb.tile([128, 1], F32)
        bv_sb = sb.tile([128, 1], F32)
        nc.sync.dma_start(bq_sb[:], bq.ap().rearrange("(p a) -> p a", a=1))
        nc.sync.dma_start(bk_sb[:], bk.ap().rearrange("(p a) -> p a", a=1))
        nc.sync.dma_start(bv_sb[:], bv.ap().rearrange("(p a) -> p a", a=1))
        bo_sb = sb.tile([1, D], BF16)
        wo_sb = sb.tile([128, 8, D], BF16)

        QT = sb.tile([128, S], BF16)      # rows 0-63 head0, 64-127 head1
        KT = sb.tile([128, S], BF16)
        # V' storage: per 128-seq block: [V_h0 (64) | 1 | V_h1 (64) | 1]
        Vp = sb.tile([128, NSB * 130], BF16)
        ones_col = sb.tile([128, 1], F32)
        nc.vector.memset(ones_col[:], 1.0)

        tri_f32 = sb.tile([128, 128], F32)  # tri[pj, j] = 1 if j >= pj else 0
        nc.gpsimd.memset(tri_f32[:], 1.0)
        nc.gpsimd.affine_select(
            out=tri_f32[:], in_=tri_f32[:], compare_op=mybir.AluOpType.is_ge,
            fill=0.0, base=0, pattern=[[1, 128]], channel_multiplier=-1)
        tri = sb.tile([128, 128], BF16)
        nc.vector.tensor_copy(tri[:], tri_f32[:])
        ident = sb.tile([128, 128], F32)
        make_identity(nc, ident[:])
        ones_row = sb.tile([1, 128], F32)
        nc.vector.memset(ones_row[:], 1.0)
        ones_sb = sb.tile([1, 128], BF16)
        nc.vector.tensor_copy(ones_sb[:], ones_row[:])

        # Output ownership: groups 0-2 span chunk pairs {0,1},{2,3},{4,5};
        # within group g rank r owns query cols [1024g+128r, 1024g+128(r+1)).
        # Chunks 6 and 7 exchange individually (64 queries per rank each) so
        # chunk 6's A2A overlaps chunk 7's attention.  Payload is the
        # producer-side-normalized O^T in bf16: rows (h*64+dk).
        a2a_in = [dram.tile([N_CORES, 128, 128], BF16, name=f"a2ain{g}")
                  for g in range(3)]
        a2a_out = [dram.tile([N_CORES, 128, 128], BF16, name=f"a2aout{g}")
                   for g in range(3)]
        a2a_in67 = [dram.tile([N_CORES, 128, 64], BF16, name=f"a2ain6{j}")
                    for j in range(2)]
        a2a_out67 = [dram.tile([N_CORES, 128, 64], BF16, name=f"a2aout6{j}")
                     for j in range(2)]

        # tiny warm-up exchange: absorbs the communicator-init barrier and
        # first-collective overhead while the early QKV chunks compute
        warm_in = dram.tile([N_CORES, 32], F32)
        warm_out = dram.tile([N_CORES, 32], F32)
        nc.gpsimd.collective_compute(
            "AllToAll", mybir.AluOpType.bypass,
            replica_groups=[list(range(N_CORES))],
            ins=[warm_in.opt()], outs=[warm_out.opt()])

        xt_r = xt.ap().rearrange("(t p) (c q) -> c p t q", p=128, q=QC)

        def make_qkv_bursts(c):
            """Per-chunk QKV work as small PE bursts. Interleaved between
            attention pairs of the previous chunk, they fill what would be
            PE idle time (keeping the HAM clock at 2.4 GHz)."""
            xt_sb = sbx.tile([128, 8, QC], BF16, tag="xt", name=f"xt{c}")
            nc.sync.dma_start(xt_sb[:], xt_r[c])
            cs = slice(c * QC, (c + 1) * QC)
            st8 = {}

            def proj_burst(w_sb, b_sb, dst):
                def run():
                    p_ps = ps_big.tile([128, 1024], F32, tag="st",
                                       name=f"qkv{c}_{dst.name}")
                    for t in range(8):
                        nc.tensor.matmul(p_ps[:, 0:512], w_sb[:, t, :],
                                         xt_sb[:, t, :],
                                         start=(t == 0), stop=(t == 7))
                    nc.vector.tensor_scalar_add(dst, p_ps[:, 0:512], b_sb[:])
                return run

            def q_burst():
                proj_burst(wq_sb, bq_sb, QT[:, cs])()
            def k_burst():
                proj_burst(wk_sb, bk_sb, KT[:, cs])()
            def v_burst():
                vt_sb = sbtmp.tile([128, QC], F32, tag="vt", name=f"vt{c}")
                st8["vt"] = vt_sb
                proj_burst(wv_sb, bv_sb, vt_sb[:])()

            def t_burst(sbk):
                def run():
                    blk = c * 4 + sbk
                    vt_sb = st8["vt"]
                    tp_ps = ps_big.tile([128, 128], F32, tag="st",
                                        name=f"tp{blk}")
                    nc.tensor.transpose(
                        tp_ps[:], vt_sb[:, sbk * 128:(sbk + 1) * 128], ident[:])
                    nc.vector.tensor_copy(Vp[:, blk * 130: blk * 130 + 64],
                                          tp_ps[:, 0:64])
                    nc.vector.tensor_copy(Vp[:, blk * 130 + 65: blk * 130 + 129],
                                          tp_ps[:, 64:128])
                    nc.vector.tensor_copy(Vp[:, blk * 130 + 64: blk * 130 + 65],
                                          ones_col[:])
                    nc.vector.tensor_copy(Vp[:, blk * 130 + 129: blk * 130 + 130],
                                          ones_col[:])
                return run

            return [q_burst, k_burst, v_burst,
                    t_burst(0), t_burst(1), t_burst(2), t_burst(3)]

        of_tiles = {}

        def make_emit_bursts(key, src_list, w, out_row0):
            """Output projection for one gathered O^T tile as PE bursts.
            src_list: list of (a2a_out tile, col offset) to gather first.
            w: stationary width (queries per emit). out_row0: first output
            row of this emit in the core's [QPER, D] output shard."""
            def gather():
                of_sb = sbn.tile([128, 8, 128], BF16, tag="of", name=f"of{key}")
                of_tiles[key] = of_sb
                for src, co in src_list:
                    nc.sync.dma_start(
                        of_sb[:, :, co:co + w],
                        src[:, :, :].rearrange("s p q -> p s q"))

            def proj(n2, co):
                def run():
                    of_sb = of_tiles[key]
                    op_ps = ps_big.tile([128, 1024], F32, tag="st",
                                        name=f"op{key}_{n2}")
                    for s in range(8):
                        nc.tensor.matmul(
                            op_ps[0:w, 0:512], of_sb[:, s, co:co + w],
                            wo_sb[:, s, n2 * 512:(n2 + 1) * 512],
                            start=(s == 0), stop=False)
                    nc.tensor.matmul(op_ps[0:w, 0:512], ones_sb[0:1, 0:w],
                                     bo_sb[0:1, n2 * 512:(n2 + 1) * 512],
                                     start=False, stop=True)
                    o_sb = sbtmp.tile([128, 512], F32, tag="osb",
                                      name=f"o{key}_{n2}")
                    nc.vector.tensor_copy(o_sb[0:w, :], op_ps[0:w, 0:512])
                    nc.sync.dma_start(
                        out.ap()[out_row0:out_row0 + w,
                                 n2 * 512:(n2 + 1) * 512],
                        o_sb[0:w, :])
                return run

            co = 0 if w == 128 else src_list[0][1]
            return [gather], [proj(0, co), proj(1, co)]

        for b in make_qkv_bursts(0):
            b()
        carry = None        # previous chunk's deferred norm+stage burst
        for c in range(SC):
            pending = [carry] if carry else []
            tail_b = []
            # emit bursts for groups whose A2A has had >=1.5 chunks to land;
            # gathers go first (DMA issue only), projections at the end
            if c == 3:
                g_b, p_b = make_emit_bursts(0, [(a2a_out[0], 0)], 128, 0)
                pending += g_b; tail_b += p_b
            if c == 5:
                g_b, p_b = make_emit_bursts(1, [(a2a_out[1], 0)], 128, 128)
                pending += g_b; tail_b += p_b
            if c == 7:
                g_b, p_b = make_emit_bursts(2, [(a2a_out[2], 0)], 128, 256)
                pending += g_b; tail_b += p_b
                g_b, p_b = make_emit_bursts(60, [(a2a_out67[0], 0)], 64, 384)
                pending += g_b; tail_b += p_b
            pending += make_qkv_bursts(c + 1) if c + 1 < SC else []
            pending += tail_b
            nb = len(pending)
            done = 0

            # ---- causal attention for chunk c, both heads ----------------
            cs = slice(c * QC, (c + 1) * QC)
            nkb = 4 * (c + 1)
            npairs = nkb // 2
            ot = ps_ot.tile([65, 1024], F32, tag="ot", name=f"ot{c}")
            ots = [ot[:, 0:512], ot[:, 512:1024]]
            for p, kbp in enumerate(range(0, nkb, 2)):
                st_h = [ps_big.tile([128, 1024], F32, tag="st",
                                    name=f"st{c}_{kbp}_{h}") for h in range(2)]
                # heads interleaved: their PE row-groups (0-63 / 64-127)
                # execute concurrently in the array
                for j in range(2):
                    kb = kbp + j
                    for h in range(2):
                        hs = slice(h * 64, (h + 1) * 64)
                        nc.tensor.matmul(
                            st_h[h][:, j * 512:(j + 1) * 512],
                            KT[hs, kb * 128:(kb + 1) * 128],
                            QT[hs, cs], start=True, stop=True)
                pt_h = []
                for h in range(2):
                    pt = sbpt.tile([128, 1024], BF16, tag="pt",
                                   name=f"pt{c}_{kbp}_{h}")
                    nc.scalar.activation(pt[:], st_h[h][:], EXP, scale=0.125)
                    for j in range(2):
                        t = kbp + j - 4 * c
                        if t >= 0:   # diagonal block: apply causal mask
                            ms = slice(j * 512 + 128 * t, j * 512 + 128 * t + 128)
                            nc.vector.tensor_mul(pt[:, ms], pt[:, ms], tri[:])
                    pt_h.append(pt)
                for h in range(2):
                    for j in range(2):
                        kb = kbp + j
                        t = kb - 4 * c
                        off = 128 * t if t > 0 else 0  # fully-masked cols skipped
                        nc.tensor.matmul(
                            ots[h][:, off:512],
                            Vp[:, kb * 130 + h * 65: kb * 130 + (h + 1) * 65],
                            pt_h[h][:, j * 512 + off:(j + 1) * 512],
                            start=(kb == 0), stop=(kb == nkb - 1))
                # spread next chunk's QKV + emit bursts across this chunk's
                # pairs
                want = (p + 1) * nb // npairs
                while done < want:
                    pending[done]()
                    done += 1
            while done < nb:
                pending[done]()
                done += 1
            if c == 1:
                nc.sync.dma_start(bo_sb[:], bo.ap().rearrange("(a n) -> a n", a=1))
                nc.sync.dma_start(wo_sb[:],
                                  wo.ap().rearrange("(t p) n -> p t n", p=128))

            # ---- producer-side softmax normalization ---------------------
            # rinv = 1/sums (row 64); broadcast across the 64 dk partitions
            # with a rank-1 matmul; normalized O^T in bf16 is the A2A payload.
            # free the ot PSUM accumulator with two short DVE copies (~2.2us,
            # under the ~3.4us HAM re-throttle window); the rest of the
            # normalization + staging is deferred into the next chunk's pair
            # loop so it never sits in the DVE FIFO ahead of the mask-muls
            # that gate the next chunk's AV matmuls.
            # NOTE: the custom-DVE reciprocal requires a partition-0 SBUF
            # input tile; feeding it a base_partition=64 slice breaks it.
            onsb = sbtmp.tile([64, 1024], F32, tag="onsb", name=f"onsb{c}")
            nc.vector.tensor_copy(onsb[:], ot[0:64, :])
            sums = sbtmp.tile([1, 1024], F32, tag="sums", name=f"sums{c}")
            nc.vector.tensor_copy(sums[:], ot[64:65, :])

            def norm_burst(c, onsb, sums):
                def run():
                    rinv = sbtmp.tile([1, 1024], F32, tag="rinv",
                                      name=f"rinv{c}")
                    nc.vector.reciprocal_approx_fast(rinv[:], sums[:])
                    rinv_b = sbtmp.tile([1, 1024], BF16, tag="rinvb",
                                        name=f"rinvb{c}")
                    nc.gpsimd.tensor_copy(rinv_b[:], rinv[:])
                    bc = ps_big.tile([128, 1024], F32, tag="st", name=f"bc{c}")
                    for half in range(2):
                        nc.tensor.matmul(
                            bc[0:64, half * 512:(half + 1) * 512],
                            ones_sb[0:1, 0:64],
                            rinv_b[0:1, half * 512:(half + 1) * 512],
                            start=True, stop=True)
                    norm = sbn.tile([64, 1024], BF16, tag="norm",
                                    name=f"norm{c}")
                    nc.vector.tensor_mul(norm[:], bc[0:64, :], onsb[:])
                    norm_h = norm[:].rearrange("p (h q) -> p h q", h=2)
                    if c < 6:
                        g, jj = c // 2, c % 2
                        for i in range(4):
                            dst = 4 * jj + i
                            nc.sync.dma_start(
                                a2a_in[g][dst, :, :]
                                .rearrange("(h p) q -> p h q", h=2),
                                norm_h[:, :, i * 128:(i + 1) * 128])
                        if jj == 1:
                            nc.gpsimd.collective_compute(
                                "AllToAll", mybir.AluOpType.bypass,
                                replica_groups=[list(range(N_CORES))],
                                ins=[a2a_in[g].opt()],
                                outs=[a2a_out[g].opt()])
                    else:
                        j67 = c - 6
                        for dst in range(8):
                            nc.sync.dma_start(
                                a2a_in67[j67][dst, :, :]
                                .rearrange("(h p) q -> p h q", h=2),
                                norm_h[:, :, dst * 64:(dst + 1) * 64])
                        nc.gpsimd.collective_compute(
                            "AllToAll", mybir.AluOpType.bypass,
                            replica_groups=[list(range(N_CORES))],
                            ins=[a2a_in67[j67].opt()],
                            outs=[a2a_out67[j67].opt()])
                return run

            if c < SC - 1:
                carry = norm_burst(c, onsb, sums)
            else:
                norm_burst(c, onsb, sums)()   # tail: run inline immediately

        # tail: only chunk 7's exchange + its half-emit remain
        g_b, p_b = make_emit_bursts(61, [(a2a_out67[1], 64)], 64, 448)
        for b in g_b + p_b:
            b()

    nc.compile()
    return nc


_NC_CACHE = {}


def _get_nc(S):
    if S not in _NC_CACHE:
        _NC_CACHE[S] = build(S)
    return _NC_CACHE[S]


def kernel(x, mask, Wq, bq, Wk, bk, Wv, bv, Wo, bo):
    import ml_dtypes
    x = np.asarray(x, np.float32)
    S = x.shape[1]
    xt = np.ascontiguousarray(x[0].T).astype(ml_dtypes.bfloat16)  # [D, S]
    Wq, Wk, Wv, Wo = (np.asarray(w, np.float32) for w in (Wq, Wk, Wv, Wo))
    bq, bk, bv, bo = (np.asarray(b, np.float32) for b in (bq, bk, bv, bo))
    # mask is structurally causal (jnp.tril in the reference); handled on-device.

    in_maps = []
    for r in range(N_CORES):
        sl = slice(128 * r, 128 * (r + 1))
        in_maps.append({
            "xt": xt,
            "wq": np.ascontiguousarray(Wq[:, sl]).astype(ml_dtypes.bfloat16),
            "wk": np.ascontiguousarray(Wk[:, sl]).astype(ml_dtypes.bfloat16),
            "wv": np.ascontiguousarray(Wv[:, sl]).astype(ml_dtypes.bfloat16),
            "wo": Wo.astype(ml_dtypes.bfloat16),
            "bq": np.ascontiguousarray(bq[sl]),
            "bk": np.ascontiguousarray(bk[sl]),
            "bv": np.ascontiguousarray(bv[sl]),
            "bo": bo.astype(ml_dtypes.bfloat16),
        })
    nc = _get_nc(S)
    global LAST_RESULT
    LAST_RESULT = run_bass_kernel_spmd(nc, in_maps, list(range(N_CORES)),
                                       trace=TRACE)
    res = LAST_RESULT.results
    full = np.empty((S, D), np.float32)
    for r in range(N_CORES):
        o = res[r]["out"]                       # [512, D]
        for g in range(3):
            full[1024 * g + 128 * r: 1024 * g + 128 * r + 128] = \
                o[128 * g: 128 * g + 128]
        full[3072 + 64 * r: 3072 + 64 * r + 64] = o[384:448]
        full[3584 + 64 * r: 3584 + 64 * r + 64] = o[448:512]
    return full[None].astype(np.float32)


TRACE = False          # test harness flips this to profile
LAST_RESULT = None
